# revision 35
# baseline (speedup 1.0000x reference)
"""Multi-head causal attention on 8 TRN2 NeuronCores (Bass/Tile).

Sharding: core = batch (2) x head-group (4 heads each). Each core computes
Q/K/V projections for its 4 heads of its batch, causal attention, and a
partial output projection (its head-slice columns of w_o). The host sums
the 4 partials per batch and adds b_o.

Matmuls run in bf16 with f32 PSUM accumulation, except the K projection
which runs in fp8e4m3 DoubleRow (2 contraction rows/cycle). The fp8
scale compensation is folded host-side into the Q weights (scores use
q*k so scaling k down and q up cancels exactly); no constants are baked
into the NEFF, so the compiled kernel stays input-independent.

v3 changes vs v2:
 - K projection fp8 DoubleRow: halves its PE time
 - K runs first: its fp8 inputs are half the bytes, so the PE prologue
   wait is shorter and the bf16 x/wq loads overlap K compute
 - after the first d-major wave, projection groups run sequentially
   (rolling PSUM reuse) instead of in waves: no wave-barrier stalls
 - wot load and ot tiles moved into phase 2 (SBUF headroom for x8)

v4 changes vs v3:
 - V projection token-tiles 4..15 interleaved into attention chunk 0,
   filling its softmax-latency bubbles (chunk 0 has no pending outproj)
 - the last token chunk runs as two 256-wide subchunks so its output
   projection overlaps the second subchunk's attention (shorter tail)
 (custom-DVE reciprocal_approx_fast was tried for the softmax reciprocal
  but this walrus build rejects CUSTOM_DVE_ANT opcodes -> stays on ACT)
"""

import os
import sys
import types
from contextlib import ExitStack

import numpy as np
import ml_dtypes

import concourse.bass as bass
import concourse.mybir as mybir
import concourse.tile as tile

BF = ml_dtypes.bfloat16
E4 = ml_dtypes.float8_e4m3
F32 = mybir.dt.float32
BF16 = mybir.dt.bfloat16
FP8 = mybir.dt.float8e4
AX = mybir.AxisListType
AF = mybir.ActivationFunctionType
DR = mybir.MatmulPerfMode.DoubleRow

P = 128          # partitions
S = 2048         # sequence length (per batch)
D = 2048         # model dim
DK = 128         # head dim
HG = 4           # heads per core
DHG = HG * DK    # 512: per-core projection width
NT = S // P      # 16 token tiles
NC = S // 512    # 4 token chunks of 512
ND = D // P      # 16 model-dim tiles
NP = ND // 2     # 8 model-dim tile PAIRS (fp8 DoubleRow contraction)
NEG = -1.0e30


def _install_ntff_hook_shim():
    """concourse's trace path imports antenv.axon_hooks, absent in this image.
    Provide it (backed by trn_agent_boot's ctypes hook when available) so
    trace=True works and trace=False never crashes on the import."""
    try:
        import antenv.axon_hooks  # noqa: F401
        return
    except ImportError:
        pass
    hook = None
    try:
        from trn_agent_boot.trn_boot import _ntff_profile_via_ctypes
        hook = _ntff_profile_via_ctypes("/opt/axon/libaxon_pjrt.so")
    except Exception:
        hook = None
    mod = types.ModuleType("antenv.axon_hooks")
    mod.get_axon_ntff_profile_hook = lambda: hook
    mod.set_axon_ntff_profile_hook = lambda h: None
    sys.modules["antenv.axon_hooks"] = mod


def _split_waits(bir_json_bytes: bytes, cap: int = 1) -> bytes:
    """walrus in this toolchain accepts at most ONE sync-wait command per
    instruction; Tile emits several. Move excess waits onto injected NoOps
    on the same engine (queues execute in order, so gating is identical)."""
    import json
    d = json.loads(bir_json_bytes)
    ctr = [0]

    def mk_nop(engine, waits):
        ctr[0] += 1
        return {
            "engine": engine, "ins": [], "outs": [],
            "name": f"I-waitfix-{ctr[0]}", "opcode": "NoOp",
            "sync_info": {"on_update": [], "on_wait": waits},
        }

    for fn in d.get("functions", []):
        for blk in fn.get("blocks", []):
            out = []
            for inst in blk.get("instructions", []):
                si = inst.get("sync_info")
                waits = (si or {}).get("on_wait", [])
                if si is not None and len(waits) > cap:
                    eng = inst["engine"]
                    extra, keep = waits[:-cap], waits[-cap:]
                    for i in range(0, len(extra), cap):
                        out.append(mk_nop(eng, extra[i:i + cap]))
                    si["on_wait"] = keep
                out.append(inst)
            blk["instructions"] = out
    return json.dumps(d).encode()


class _FixedBass(bass.Bass):
    def to_json_bytes(self):
        return _split_waits(super().to_json_bytes(), cap=1)


def build_bass() -> bass.Bass:
    nc = _FixedBass()

    # xt: [D, S] model-major; a per-d row band [128, 2048] is one
    # contiguous 512KB block (4KB per partition line).
    xt = nc.declare_dram_parameter("xt", [D, S], BF16, isOutput=False)
    # x8: fp8 x^T, pair-major: x8[pair, p, i, :] = x^T[(2*pair+i)*128+p, :]
    x8 = nc.declare_dram_parameter("x8", [NP, P, 2, S], FP8, isOutput=False)
    # wk8[p, d2, i, j] = wk^T[(2*d2+i)*128+p, j] (scaled to fp8 range)
    wk8 = nc.declare_dram_parameter("wk8", [P, NP, 2, DHG], FP8, isOutput=False)
    # bf16 weights partition-major: w2[p, d*DHG + j] = w[d*P + p, j]
    wq2 = nc.declare_dram_parameter("wq2", [P, ND * DHG], BF16, isOutput=False)
    wv2 = nc.declare_dram_parameter("wv2", [P, ND * DHG], BF16, isOutput=False)
    # wot2[p, h*S + m] = w_o[m, c*DHG + h*P + p]
    wot2 = nc.declare_dram_parameter("wot2", [P, HG * S], BF16, isOutput=False)
    bqt = nc.declare_dram_parameter("bqt", [P, HG], F32, isOutput=False)
    bkt = nc.declare_dram_parameter("bkt", [P, HG], F32, isOutput=False)
    bvb = nc.declare_dram_parameter("bvb", [P, DHG], F32, isOutput=False)
    dmask = nc.declare_dram_parameter("dmask", [P, P], F32, isOutput=False)
    # out tiles stored contiguously per (m, chunk) block: DMA writes one
    # 128KB contiguous region each (the host re-tiles to [S, D])
    out = nc.declare_dram_parameter("out", [ND, NC, P, 512], BF16,
                                    isOutput=True)

    with tile.TileContext(nc) as tc, ExitStack() as ctx:
        # ---- persistent tiles ----
        const = ctx.enter_context(tc.tile_pool(name="const", bufs=1))
        act = ctx.enter_context(tc.tile_pool(name="act", bufs=1))

        qt_sb = [act.tile([P, S], BF16, name=f"qt{h}") for h in range(HG)]
        kt_sb = [act.tile([P, S], BF16, name=f"kt{h}") for h in range(HG)]
        v_sb = [act.tile([P, DHG], BF16, name=f"v{t}") for t in range(NT)]
        # xt/wv stay allocated into phase 2: V token-tiles 4..15 are
        # issued between attention chunk-0 head units (PE bubble fill).
        xt_sb = [act.tile([P, S], BF16, name=f"x{d}") for d in range(ND)]
        wv_sb = act.tile([P, ND * DHG], BF16, name="wvall")

        bq_sb = const.tile([P, HG], F32, name="bq")
        bk_sb = const.tile([P, HG], F32, name="bk")
        bv_sb = const.tile([P, DHG], F32, name="bv")
        mask_sb = const.tile([P, P], F32, name="mask")
        ones_sb = const.tile([P, P], BF16, name="ones")

        with ExitStack() as p1:
            xp = p1.enter_context(tc.tile_pool(name="xp", bufs=1))
            ps1 = p1.enter_context(tc.tile_pool(name="ps1", bufs=8, space="PSUM"))

            # ---- DMA issue order = first-use order ----
            wk8_sb = xp.tile([P, NP, 2, DHG], FP8, name="wk8all")
            nc.sync.dma_start(wk8_sb[:, 0:1, :, :], wk8[:, 0:1, :, :])
            x8_sb = []
            for pr in range(NP):
                t_ = xp.tile([P, 2, S], FP8, name=f"x8_{pr}")
                if pr == 0:
                    # first tile in col-chunks so the first K wave starts
                    # as soon as chunk 0 lands
                    for cq in range(4):
                        nc.sync.dma_start(t_[:, :, cq * 512:(cq + 1) * 512],
                                          x8[pr, :, :, cq * 512:(cq + 1) * 512])
                else:
                    # split by the pair dim: a DMA's time is its per-partition
                    # packet count on ONE queue, so two half-tile DMAs on two
                    # queues land the tile in half the time
                    nc.sync.dma_start(t_[:, 0, :], x8[pr, :, 0, :])
                    nc.sync.dma_start(t_[:, 1, :], x8[pr, :, 1, :])
                x8_sb.append(t_)
                if pr == 0:
                    nc.sync.dma_start(wk8_sb[:, 1:4, :, :], wk8[:, 1:4, :, :])
                if pr == 1:
                    nc.sync.dma_start(wk8_sb[:, 4:8, :, :], wk8[:, 4:8, :, :])
            nc.sync.dma_start(bk_sb[:], bkt[:, :])
            wq_sb = xp.tile([P, ND * DHG], BF16, name="wqall")
            WCH = 4 * DHG
            for c4 in range(ND // 4):
                nc.sync.dma_start(wq_sb[:, c4 * WCH:(c4 + 1) * WCH],
                                  wq2[:, c4 * WCH:(c4 + 1) * WCH])
            nc.sync.dma_start(bq_sb[:], bqt[:, :])
            for d in range(ND):
                nc.sync.dma_start(xt_sb[d][:], xt[d * P:(d + 1) * P, :])
                if d == 7:
                    for c4 in range(ND // 4):
                        nc.sync.dma_start(wv_sb[:, c4 * WCH:(c4 + 1) * WCH],
                                          wv2[:, c4 * WCH:(c4 + 1) * WCH])
                    nc.sync.dma_start(bv_sb[:], bvb[:, :])
                if d == 11:
                    nc.sync.dma_start(mask_sb[:], dmask[:, :])
                    nc.vector.memset(ones_sb[:], 1.0)

            # ---- phase 1: K (fp8 DoubleRow) first, then Q, then V[0..3] ----
            # First wave is pair-major (each contraction round needs only
            # x8[pr]) so the PE streams along with the initial DMAs; later
            # groups run sequentially on rolling PSUM banks.
            kgroups = [(h, c) for h in range(HG) for c in range(NC)]
            pss = []
            for _ in range(8):
                pss.append(ps1.tile([P, 512], F32, name="p1"))
            for pr in range(NP):
                for j, (h, c) in enumerate(kgroups[:8]):
                    nc.tensor.matmul(
                        pss[j][:], wk8_sb[:, pr, :, h * P:(h + 1) * P],
                        x8_sb[pr][:, :, c * 512:(c + 1) * 512],
                        start=(pr == 0), stop=(pr == NP - 1),
                        perf_mode=DR)
            for j, (h, c) in enumerate(kgroups[:8]):
                nc.scalar.activation(kt_sb[h][:, c * 512:(c + 1) * 512],
                                     pss[j][:], AF.Identity,
                                     bias=bk_sb[:, h:h + 1])
            for (h, c) in kgroups[8:]:
                pk = ps1.tile([P, 512], F32, name="p1")
                for pr in range(NP):
                    nc.tensor.matmul(
                        pk[:], wk8_sb[:, pr, :, h * P:(h + 1) * P],
                        x8_sb[pr][:, :, c * 512:(c + 1) * 512],
                        start=(pr == 0), stop=(pr == NP - 1),
                        perf_mode=DR)
                nc.scalar.activation(kt_sb[h][:, c * 512:(c + 1) * 512],
                                     pk[:], AF.Identity,
                                     bias=bk_sb[:, h:h + 1])
            for (h, c) in kgroups:
                pq = ps1.tile([P, 512], F32, name="p1")
                for d in range(ND):
                    nc.tensor.matmul(
                        pq[:], wq_sb[:, d * DHG + h * P:d * DHG + (h + 1) * P],
                        xt_sb[d][:, c * 512:(c + 1) * 512],
                        start=(d == 0), stop=(d == ND - 1))
                nc.scalar.activation(qt_sb[h][:, c * 512:(c + 1) * 512],
                                     pq[:], AF.Identity,
                                     bias=bq_sb[:, h:h + 1])
            for t in range(4):
                pv = ps1.tile([P, 512], F32, name="p1")
                for d in range(ND):
                    nc.tensor.matmul(
                        pv[:], xt_sb[d][:, t * P:(t + 1) * P],
                        wv_sb[:, d * DHG:(d + 1) * DHG],
                        start=(d == 0), stop=(d == ND - 1))
                nc.vector.tensor_add(v_sb[t][:], pv[:], bv_sb[:])

        # ---- phase 2+3: causal attention per head, fused output proj ----
        # Scores are computed TRANSPOSED (S^T[k, q]) so exp() writes the AV
        # moving operand directly -- no PE transposes, no PSUM round-trip.
        # Row sums come from ones-vector matmuls accumulated alongside AV;
        # normalization happens once per [dv, q-chunk] at OT eviction.
        with ExitStack() as p2:
            # PSUM budget (8 banks): sp 2x2 (paired score tiles) + otp 1
            # (the interleave slots cover po's eviction latency) + smps 1
            # + ps3 2 (outproj/vgroup double-buffer)
            sp = p2.enter_context(tc.tile_pool(name="sp", bufs=2, space="PSUM"))
            otp = p2.enter_context(tc.tile_pool(name="otp", bufs=1, space="PSUM"))
            smps = p2.enter_context(tc.tile_pool(name="smps", bufs=1, space="PSUM"))
            ps3 = p2.enter_context(tc.tile_pool(name="ps3", bufs=2, space="PSUM"))
            pp = p2.enter_context(tc.tile_pool(name="pp", bufs=4))
            smp = p2.enter_context(tc.tile_pool(name="smp", bufs=2))
            ost = p2.enter_context(tc.tile_pool(name="ost", bufs=3))
            stp = p2.enter_context(tc.tile_pool(name="stp", bufs=1))
            wop = p2.enter_context(tc.tile_pool(name="wop", bufs=1))

            # wot + ot live where the freed x tiles were; the wot DMA lands
            # during attention of chunk 0, well before outproj(0) needs it.
            wot_sb = wop.tile([P, HG * S], BF16, name="wotall")
            for h2 in range(HG // 2):
                nc.sync.dma_start(wot_sb[:, h2 * 2 * S:(h2 + 1) * 2 * S],
                                  wot2[:, h2 * 2 * S:(h2 + 1) * 2 * S])
            ot_sb = [wop.tile([P, S], BF16, name=f"ot{h}") for h in range(HG)]

            def vgroup(t):
                """deferred V projection token-tile t (PE bubble fill)"""
                pv = ps3.tile([P, 512], F32, name="ps3t")
                for d in range(ND):
                    nc.tensor.matmul(
                        pv[:], xt_sb[d][:, t * P:(t + 1) * P],
                        wv_sb[:, d * DHG:(d + 1) * DHG],
                        start=(d == 0), stop=(d == ND - 1))
                nc.vector.tensor_add(v_sb[t][:], pv[:], bv_sb[:])

            def outproj(q0, w, ms):
                """output projection of token cols [q0, q0+w) for m-tiles ms"""
                for m in ms:
                    ps = ps3.tile([P, 512], F32, name="ps3t")
                    for h in range(HG):
                        nc.tensor.matmul(
                            ps[:, :w], wot_sb[:, h * S + m * P:h * S + (m + 1) * P],
                            ot_sb[h][:, q0:q0 + w],
                            start=(h == 0), stop=(h == HG - 1))
                    st = ost.tile([P, 512], BF16, name="st")
                    nc.vector.tensor_copy(st[:, :w], ps[:, :w])
                    nc.sync.dma_start(
                        out[m, q0 // 512, :, :w], st[:, :w])

            def attn_unit(h, q0, w):
                """causal attention for head h, query cols [q0, q0+w).

                Key tiles processed in PAIRS on 2-bank score tiles: both
                sub-diagonal tiles of a pair share ONE exp (halves ACT's
                352-elem fixed cost); AV/sums of pair u-1 are issued after
                pair u's scores so the PE pipelines across the exp latency.
                """
                nkt = (q0 + w) // P          # key tiles 0..nkt-1; always even
                po = otp.tile([P, 512], F32, name="po")
                sums = smps.tile([P, 512], F32, name="sums")
                pend = []                    # (kt, pcp, region, off) to flush

                def flush():
                    for (kt, pcp, reg, off) in pend:
                        nc.tensor.matmul(
                            po[:, off:w], v_sb[kt][:, h * P:(h + 1) * P],
                            pcp[:, reg, off:w],
                            start=(kt == 0), stop=(kt == nkt - 1),
                            skip_group_check=True)
                        nc.tensor.matmul(
                            sums[:, off:w], ones_sb[:], pcp[:, reg, off:w],
                            start=(kt == 0), stop=(kt == nkt - 1),
                            skip_group_check=True)
                    pend.clear()

                for u in range(nkt // 2):
                    k0, k1 = 2 * u, 2 * u + 1
                    ko0, ko1 = k0 * P - q0, k1 * P - q0
                    m0, m1 = max(0, ko0), max(0, ko1)
                    spb = sp.tile([P, 2, 512], F32, name="ps")
                    nc.tensor.matmul(
                        spb[:, 0, m0:w], kt_sb[h][:, k0 * P:(k0 + 1) * P],
                        qt_sb[h][:, q0 + m0:q0 + w], start=True, stop=True)
                    nc.tensor.matmul(
                        spb[:, 1, m1:w], kt_sb[h][:, k1 * P:(k1 + 1) * P],
                        qt_sb[h][:, q0 + m1:q0 + w], start=True, stop=True)
                    pcp = pp.tile([P, 2, 512], BF16, name="pcp")
                    if ko1 < 0:
                        # both sub-diagonal: one merged exp for the pair
                        nc.scalar.activation(pcp[:, :, :w], spb[:, :, :w],
                                             AF.Exp)
                    else:
                        # diagonal band: triangular mask per tile, then
                        # separate exps over the unmasked column ranges
                        for reg, ko in ((0, ko0), (1, ko1)):
                            off = max(0, ko)
                            if ko >= 0:
                                nc.vector.tensor_add(
                                    spb[:, reg, ko:ko + P],
                                    spb[:, reg, ko:ko + P], mask_sb[:])
                            nc.scalar.activation(pcp[:, reg, off:w],
                                                 spb[:, reg, off:w], AF.Exp)
                    flush()
                    pend.append((k0, pcp, 0, m0))
                    pend.append((k1, pcp, 1, m1))
                flush()
                # 1/x as exp(-ln(x)) on ACT: ~1.3us vs 3.4us DVE divide,
                # and off the DVE critical path (sums are always > 0).
                lg = smp.tile([P, 512], F32, name="lg")
                nc.scalar.activation(lg[:, :w], sums[:, :w], AF.Ln)
                rec = smp.tile([P, 512], F32, name="rec")
                nc.scalar.activation(rec[:, :w], lg[:, :w], AF.Exp, scale=-1.0)
                nc.vector.tensor_mul(ot_sb[h][:, q0:q0 + w],
                                     po[:, :w], rec[:, :w])

            # g-major, h-minor: adjacent (h,g) units are independent heads, so
            # the PE always has a second stream to fill softmax-latency gaps.
            # Between head units: chunk 0 fills with deferred V tiles, later
            # chunks with the previous chunk's output projection.
            for g in range(NC - 1):
                for h in range(HG):
                    attn_unit(h, g * 512, 512)
                    if g == 0:
                        for t in range(4 + 3 * h, 7 + 3 * h):
                            vgroup(t)
                    else:
                        outproj((g - 1) * 512, 512, range(h * 4, (h + 1) * 4))
            # last chunk as two 256-wide subchunks: the first subchunk's
            # output projection overlaps the second subchunk's attention.
            # Both subchunks' results collect in persistent [P,512] tiles;
            # one fat DMA per m-tile at the end (1KB lines, fewer packets).
            stf = [stp.tile([P, 512], BF16, name=f"stf{m}") for m in range(ND)]

            def outproj_tail(q0, half, ms):
                for m in ms:
                    ps = ps3.tile([P, 512], F32, name="ps3t")
                    for h in range(HG):
                        nc.tensor.matmul(
                            ps[:, :256],
                            wot_sb[:, h * S + m * P:h * S + (m + 1) * P],
                            ot_sb[h][:, q0:q0 + 256],
                            start=(h == 0), stop=(h == HG - 1))
                    nc.vector.tensor_copy(
                        stf[m][:, half * 256:(half + 1) * 256], ps[:, :256])
                    if half == 1:
                        nc.sync.dma_start(out[m, 3, :, :], stf[m][:])

            for h in range(HG):
                attn_unit(h, 1536, 256)
                outproj(1024, 512, range(h * 4, (h + 1) * 4))
            # chunk-3a's projection is fully ready once 3a's last head
            # normalizes; pull a group AHEAD of 3b's attention so the
            # subchunk boundary gets extra queued PE work to bridge the
            # ACT/DVE backlog there, while keeping fill at every later
            # head boundary (incl. before the final projection).
            slots = [range(0, 4), range(4, 8), range(8, 11),
                     range(11, 14), range(14, 16)]
            outproj_tail(1536, 0, slots[0])
            for h in range(HG):
                attn_unit(h, 1792, 256)
                outproj_tail(1536, 0, slots[h + 1])
            outproj_tail(1792, 1, range(ND))

    return nc


_NC_CACHE = None


def _get_nc():
    global _NC_CACHE
    if _NC_CACHE is None:
        _NC_CACHE = build_bass()
    return _NC_CACHE


def _prep_core_inputs(x, w_q, b_q, w_k, b_k, w_v, b_v, w_o, b_o, b, c):
    """Host-side shard prep for core (batch b, head-group c)."""
    hsl = slice(c * DHG, (c + 1) * DHG)
    scale = np.float32(1.0 / np.sqrt(DK))

    def pmajor(wt):
        # wt: [D, DHG] (model-major) -> [P, ND*DHG] partition-major
        return np.ascontiguousarray(
            wt.reshape(ND, P, DHG).transpose(1, 0, 2).reshape(P, ND * DHG))

    xb = x[b].T                       # [D, S] f32
    wkt = w_k[hsl].T                  # [D, DHG] f32
    # fp8 scaling: K path computes (x/sx)@(wk/sw); the sx*sw factor is
    # folded into the Q weights/bias (scores = q^T k is scale-invariant)
    # and into b_k (added to the scaled psum at eviction).
    sx = np.float32(np.abs(xb).max() / 448.0 * 2.0)
    sw = np.float32(np.abs(wkt).max() / 448.0 * 2.0)
    sxw = np.float32(sx * sw)

    xtn = np.ascontiguousarray(xb).astype(BF)
    x8n = np.ascontiguousarray(
        (xb / sx).astype(E4).reshape(NP, 2, P, S).transpose(0, 2, 1, 3))
    wk8n = np.ascontiguousarray(
        (wkt / sw).astype(E4).reshape(NP, 2, P, DHG).transpose(2, 0, 1, 3))
    wqtn = pmajor((w_q[hsl] * (scale * sxw)).T.astype(BF))
    wvtn = pmajor(w_v[hsl].T.astype(BF))
    # w_o slice: [DHG, D]; wot2[p, h*S + m] = w_o[m, c*DHG + h*P + p]
    wotn = np.ascontiguousarray(
        w_o[:, hsl].T.astype(BF).reshape(HG, P, D).transpose(1, 0, 2)
        .reshape(P, HG * D))
    bqtn = np.ascontiguousarray(
        (b_q[hsl] * (scale * sxw)).reshape(HG, P).T).astype(np.float32)
    bktn = np.ascontiguousarray(
        (b_k[hsl] / sxw).reshape(HG, P).T).astype(np.float32)
    bvbn = np.ascontiguousarray(np.tile(b_v[hsl], (P, 1))).astype(np.float32)
    i = np.arange(P)[:, None]
    j = np.arange(P)[None, :]
    dmaskn = np.where(j >= i, np.float32(0.0), np.float32(NEG)).astype(np.float32)
    return {
        "xt": xtn, "x8": x8n, "wk8": wk8n, "wq2": wqtn, "wv2": wvtn,
        "wot2": wotn, "bqt": bqtn, "bkt": bktn, "bvb": bvbn, "dmask": dmaskn,
    }


def kernel(x, w_q, b_q, w_k, b_k, w_v, b_v, w_o, b_o, *,
           _trace=False, _tmpdir=None):
    _install_ntff_hook_shim()
    from concourse.bass_utils import run_bass_kernel_spmd

    x = np.asarray(x, dtype=np.float32)
    w_q = np.asarray(w_q, dtype=np.float32)
    b_q = np.asarray(b_q, dtype=np.float32)
    w_k = np.asarray(w_k, dtype=np.float32)
    b_k = np.asarray(b_k, dtype=np.float32)
    w_v = np.asarray(w_v, dtype=np.float32)
    b_v = np.asarray(b_v, dtype=np.float32)
    w_o = np.asarray(w_o, dtype=np.float32)
    b_o = np.asarray(b_o, dtype=np.float32)

    nc = _get_nc()
    in_maps = []
    for core in range(8):
        b, c = divmod(core, 4)
        in_maps.append(_prep_core_inputs(x, w_q, b_q, w_k, b_k, w_v, b_v,
                                         w_o, b_o, b, c))
    kwargs = {}
    if _trace:
        kwargs.update(trace=True, tmpdir=_tmpdir)
    res = run_bass_kernel_spmd(nc, in_maps, core_ids=list(range(8)), **kwargs)

    B = x.shape[0]
    outp = np.zeros((B, S, D), dtype=np.float32)
    for core in range(8):
        b, c = divmod(core, 4)
        blk = res.results[core]["out"]  # [ND, NC, P, 512] tile blocks
        full = blk.transpose(0, 2, 1, 3).reshape(D, S)
        outp[b] += full.T.astype(np.float32)
    outp += b_o[None, None, :]
    kernel.last_results = res
    return outp


# revision 36
# speedup vs baseline: 1.0031x; 1.0031x over previous
"""Multi-head causal attention on 8 TRN2 NeuronCores (Bass/Tile).

Sharding: core = batch (2) x head-group (4 heads each). Each core computes
Q/K/V projections for its 4 heads of its batch, causal attention, and a
partial output projection (its head-slice columns of w_o). The host sums
the 4 partials per batch and adds b_o.

Matmuls run in bf16 with f32 PSUM accumulation, except the K projection
which runs in fp8e4m3 DoubleRow (2 contraction rows/cycle). The fp8
scale compensation is folded host-side into the Q weights (scores use
q*k so scaling k down and q up cancels exactly); no constants are baked
into the NEFF, so the compiled kernel stays input-independent.

v3 changes vs v2:
 - K projection fp8 DoubleRow: halves its PE time
 - K runs first: its fp8 inputs are half the bytes, so the PE prologue
   wait is shorter and the bf16 x/wq loads overlap K compute
 - after the first d-major wave, projection groups run sequentially
   (rolling PSUM reuse) instead of in waves: no wave-barrier stalls
 - wot load and ot tiles moved into phase 2 (SBUF headroom for x8)

v4 changes vs v3:
 - V projection token-tiles 4..15 interleaved into attention chunk 0,
   filling its softmax-latency bubbles (chunk 0 has no pending outproj)
 - the last token chunk runs as two 256-wide subchunks so its output
   projection overlaps the second subchunk's attention (shorter tail)
 (custom-DVE reciprocal_approx_fast was tried for the softmax reciprocal
  but this walrus build rejects CUSTOM_DVE_ANT opcodes -> stays on ACT)
"""

import os
import sys
import types
from contextlib import ExitStack

import numpy as np
import ml_dtypes

import concourse.bass as bass
import concourse.mybir as mybir
import concourse.tile as tile

BF = ml_dtypes.bfloat16
E4 = ml_dtypes.float8_e4m3
F32 = mybir.dt.float32
BF16 = mybir.dt.bfloat16
FP8 = mybir.dt.float8e4
AX = mybir.AxisListType
AF = mybir.ActivationFunctionType
DR = mybir.MatmulPerfMode.DoubleRow

P = 128          # partitions
S = 2048         # sequence length (per batch)
D = 2048         # model dim
DK = 128         # head dim
HG = 4           # heads per core
DHG = HG * DK    # 512: per-core projection width
NT = S // P      # 16 token tiles
NC = S // 512    # 4 token chunks of 512
ND = D // P      # 16 model-dim tiles
NP = ND // 2     # 8 model-dim tile PAIRS (fp8 DoubleRow contraction)
NEG = -1.0e30


def _install_ntff_hook_shim():
    """concourse's trace path imports antenv.axon_hooks, absent in this image.
    Provide it (backed by trn_agent_boot's ctypes hook when available) so
    trace=True works and trace=False never crashes on the import."""
    try:
        import antenv.axon_hooks  # noqa: F401
        return
    except ImportError:
        pass
    hook = None
    try:
        from trn_agent_boot.trn_boot import _ntff_profile_via_ctypes
        hook = _ntff_profile_via_ctypes("/opt/axon/libaxon_pjrt.so")
    except Exception:
        hook = None
    mod = types.ModuleType("antenv.axon_hooks")
    mod.get_axon_ntff_profile_hook = lambda: hook
    mod.set_axon_ntff_profile_hook = lambda h: None
    sys.modules["antenv.axon_hooks"] = mod


def _split_waits(bir_json_bytes: bytes, cap: int = 1) -> bytes:
    """walrus in this toolchain accepts at most ONE sync-wait command per
    instruction; Tile emits several. Move excess waits onto injected NoOps
    on the same engine (queues execute in order, so gating is identical)."""
    import json
    d = json.loads(bir_json_bytes)
    ctr = [0]

    def mk_nop(engine, waits):
        ctr[0] += 1
        return {
            "engine": engine, "ins": [], "outs": [],
            "name": f"I-waitfix-{ctr[0]}", "opcode": "NoOp",
            "sync_info": {"on_update": [], "on_wait": waits},
        }

    for fn in d.get("functions", []):
        for blk in fn.get("blocks", []):
            out = []
            for inst in blk.get("instructions", []):
                si = inst.get("sync_info")
                waits = (si or {}).get("on_wait", [])
                if si is not None and len(waits) > cap:
                    eng = inst["engine"]
                    extra, keep = waits[:-cap], waits[-cap:]
                    for i in range(0, len(extra), cap):
                        out.append(mk_nop(eng, extra[i:i + cap]))
                    si["on_wait"] = keep
                out.append(inst)
            blk["instructions"] = out
    return json.dumps(d).encode()


class _FixedBass(bass.Bass):
    def to_json_bytes(self):
        return _split_waits(super().to_json_bytes(), cap=1)


def build_bass() -> bass.Bass:
    nc = _FixedBass()

    # xt: [D, S] model-major; a per-d row band [128, 2048] is one
    # contiguous 512KB block (4KB per partition line).
    xt = nc.declare_dram_parameter("xt", [D, S], BF16, isOutput=False)
    # x8: fp8 x^T, pair-major: x8[pair, p, i, :] = x^T[(2*pair+i)*128+p, :]
    x8 = nc.declare_dram_parameter("x8", [NP, P, 2, S], FP8, isOutput=False)
    # wk8[p, d2, i, j] = wk^T[(2*d2+i)*128+p, j] (scaled to fp8 range)
    wk8 = nc.declare_dram_parameter("wk8", [P, NP, 2, DHG], FP8, isOutput=False)
    # bf16 weights partition-major: w2[p, d*DHG + j] = w[d*P + p, j]
    wq2 = nc.declare_dram_parameter("wq2", [P, ND * DHG], BF16, isOutput=False)
    wv2 = nc.declare_dram_parameter("wv2", [P, ND * DHG], BF16, isOutput=False)
    # wot2[p, h*S + m] = w_o[m, c*DHG + h*P + p]
    wot2 = nc.declare_dram_parameter("wot2", [P, HG * S], BF16, isOutput=False)
    bqt = nc.declare_dram_parameter("bqt", [P, HG], F32, isOutput=False)
    bkt = nc.declare_dram_parameter("bkt", [P, HG], F32, isOutput=False)
    bvb = nc.declare_dram_parameter("bvb", [P, DHG], F32, isOutput=False)
    dmask = nc.declare_dram_parameter("dmask", [P, P], F32, isOutput=False)
    # out tiles stored contiguously per (m, chunk) block: DMA writes one
    # 128KB contiguous region each (the host re-tiles to [S, D])
    out = nc.declare_dram_parameter("out", [ND, NC, P, 512], BF16,
                                    isOutput=True)

    with tile.TileContext(nc) as tc, ExitStack() as ctx:
        # ---- persistent tiles ----
        const = ctx.enter_context(tc.tile_pool(name="const", bufs=1))
        act = ctx.enter_context(tc.tile_pool(name="act", bufs=1))

        qt_sb = [act.tile([P, S], BF16, name=f"qt{h}") for h in range(HG)]
        kt_sb = [act.tile([P, S], BF16, name=f"kt{h}") for h in range(HG)]
        v_sb = [act.tile([P, DHG], BF16, name=f"v{t}") for t in range(NT)]
        # xt/wv stay allocated into phase 2: V token-tiles 4..15 are
        # issued between attention chunk-0 head units (PE bubble fill).
        xt_sb = [act.tile([P, S], BF16, name=f"x{d}") for d in range(ND)]
        wv_sb = act.tile([P, ND * DHG], BF16, name="wvall")

        bq_sb = const.tile([P, HG], F32, name="bq")
        bk_sb = const.tile([P, HG], F32, name="bk")
        bv_sb = const.tile([P, DHG], F32, name="bv")
        mask_sb = const.tile([P, P], F32, name="mask")
        ones_sb = const.tile([P, P], BF16, name="ones")

        with ExitStack() as p1:
            xp = p1.enter_context(tc.tile_pool(name="xp", bufs=1))
            ps1 = p1.enter_context(tc.tile_pool(name="ps1", bufs=8, space="PSUM"))

            # ---- DMA issue order = first-use order ----
            wk8_sb = xp.tile([P, NP, 2, DHG], FP8, name="wk8all")
            nc.sync.dma_start(wk8_sb[:, 0:1, :, :], wk8[:, 0:1, :, :])
            x8_sb = []
            for pr in range(NP):
                t_ = xp.tile([P, 2, S], FP8, name=f"x8_{pr}")
                if pr == 0:
                    # first tile in col-chunks so the first K wave starts
                    # as soon as chunk 0 lands
                    for cq in range(4):
                        nc.sync.dma_start(t_[:, :, cq * 512:(cq + 1) * 512],
                                          x8[pr, :, :, cq * 512:(cq + 1) * 512])
                else:
                    nc.sync.dma_start(t_[:], x8[pr, :, :, :])
                x8_sb.append(t_)
                if pr == 0:
                    nc.sync.dma_start(wk8_sb[:, 1:4, :, :], wk8[:, 1:4, :, :])
                if pr == 1:
                    nc.sync.dma_start(wk8_sb[:, 4:8, :, :], wk8[:, 4:8, :, :])
            nc.sync.dma_start(bk_sb[:], bkt[:, :])
            wq_sb = xp.tile([P, ND * DHG], BF16, name="wqall")
            WCH = 4 * DHG
            for c4 in range(ND // 4):
                nc.sync.dma_start(wq_sb[:, c4 * WCH:(c4 + 1) * WCH],
                                  wq2[:, c4 * WCH:(c4 + 1) * WCH])
            nc.sync.dma_start(bq_sb[:], bqt[:, :])
            for d in range(ND):
                nc.sync.dma_start(xt_sb[d][:], xt[d * P:(d + 1) * P, :])
                if d == 7:
                    for c4 in range(ND // 4):
                        nc.sync.dma_start(wv_sb[:, c4 * WCH:(c4 + 1) * WCH],
                                          wv2[:, c4 * WCH:(c4 + 1) * WCH])
                    nc.sync.dma_start(bv_sb[:], bvb[:, :])
                if d == 11:
                    nc.sync.dma_start(mask_sb[:], dmask[:, :])
                    nc.vector.memset(ones_sb[:], 1.0)

            # ---- phase 1: K (fp8 DoubleRow) first, then Q, then V[0..3] ----
            # First wave is pair-major (each contraction round needs only
            # x8[pr]) so the PE streams along with the initial DMAs; later
            # groups run sequentially on rolling PSUM banks.
            kgroups = [(h, c) for h in range(HG) for c in range(NC)]
            pss = []
            for _ in range(8):
                pss.append(ps1.tile([P, 512], F32, name="p1"))
            for pr in range(NP):
                for j, (h, c) in enumerate(kgroups[:8]):
                    nc.tensor.matmul(
                        pss[j][:], wk8_sb[:, pr, :, h * P:(h + 1) * P],
                        x8_sb[pr][:, :, c * 512:(c + 1) * 512],
                        start=(pr == 0), stop=(pr == NP - 1),
                        perf_mode=DR)
            for j, (h, c) in enumerate(kgroups[:8]):
                nc.scalar.activation(kt_sb[h][:, c * 512:(c + 1) * 512],
                                     pss[j][:], AF.Identity,
                                     bias=bk_sb[:, h:h + 1])
            for (h, c) in kgroups[8:]:
                pk = ps1.tile([P, 512], F32, name="p1")
                for pr in range(NP):
                    nc.tensor.matmul(
                        pk[:], wk8_sb[:, pr, :, h * P:(h + 1) * P],
                        x8_sb[pr][:, :, c * 512:(c + 1) * 512],
                        start=(pr == 0), stop=(pr == NP - 1),
                        perf_mode=DR)
                nc.scalar.activation(kt_sb[h][:, c * 512:(c + 1) * 512],
                                     pk[:], AF.Identity,
                                     bias=bk_sb[:, h:h + 1])
            for (h, c) in kgroups:
                pq = ps1.tile([P, 512], F32, name="p1")
                for d in range(ND):
                    nc.tensor.matmul(
                        pq[:], wq_sb[:, d * DHG + h * P:d * DHG + (h + 1) * P],
                        xt_sb[d][:, c * 512:(c + 1) * 512],
                        start=(d == 0), stop=(d == ND - 1))
                nc.scalar.activation(qt_sb[h][:, c * 512:(c + 1) * 512],
                                     pq[:], AF.Identity,
                                     bias=bq_sb[:, h:h + 1])
            for t in range(4):
                pv = ps1.tile([P, 512], F32, name="p1")
                for d in range(ND):
                    nc.tensor.matmul(
                        pv[:], xt_sb[d][:, t * P:(t + 1) * P],
                        wv_sb[:, d * DHG:(d + 1) * DHG],
                        start=(d == 0), stop=(d == ND - 1))
                nc.vector.tensor_add(v_sb[t][:], pv[:], bv_sb[:])

        # ---- phase 2+3: causal attention per head, fused output proj ----
        # Scores are computed TRANSPOSED (S^T[k, q]) so exp() writes the AV
        # moving operand directly -- no PE transposes, no PSUM round-trip.
        # Row sums come from ones-vector matmuls accumulated alongside AV;
        # normalization happens once per [dv, q-chunk] at OT eviction.
        with ExitStack() as p2:
            # PSUM budget (8 banks): sp 2x2 (paired score tiles) + otp 1
            # (the interleave slots cover po's eviction latency) + smps 1
            # + ps3 2 (outproj/vgroup double-buffer)
            sp = p2.enter_context(tc.tile_pool(name="sp", bufs=2, space="PSUM"))
            otp = p2.enter_context(tc.tile_pool(name="otp", bufs=1, space="PSUM"))
            smps = p2.enter_context(tc.tile_pool(name="smps", bufs=1, space="PSUM"))
            ps3 = p2.enter_context(tc.tile_pool(name="ps3", bufs=2, space="PSUM"))
            pp = p2.enter_context(tc.tile_pool(name="pp", bufs=4))
            smp = p2.enter_context(tc.tile_pool(name="smp", bufs=2))
            ost = p2.enter_context(tc.tile_pool(name="ost", bufs=3))
            stp = p2.enter_context(tc.tile_pool(name="stp", bufs=1))
            wop = p2.enter_context(tc.tile_pool(name="wop", bufs=1))

            # wot + ot live where the freed x tiles were; the wot DMA lands
            # during attention of chunk 0, well before outproj(0) needs it.
            wot_sb = wop.tile([P, HG * S], BF16, name="wotall")
            for h2 in range(HG // 2):
                nc.sync.dma_start(wot_sb[:, h2 * 2 * S:(h2 + 1) * 2 * S],
                                  wot2[:, h2 * 2 * S:(h2 + 1) * 2 * S])
            ot_sb = [wop.tile([P, S], BF16, name=f"ot{h}") for h in range(HG)]

            def vgroup(t):
                """deferred V projection token-tile t (PE bubble fill)"""
                pv = ps3.tile([P, 512], F32, name="ps3t")
                for d in range(ND):
                    nc.tensor.matmul(
                        pv[:], xt_sb[d][:, t * P:(t + 1) * P],
                        wv_sb[:, d * DHG:(d + 1) * DHG],
                        start=(d == 0), stop=(d == ND - 1))
                nc.vector.tensor_add(v_sb[t][:], pv[:], bv_sb[:])

            def outproj(q0, w, ms):
                """output projection of token cols [q0, q0+w) for m-tiles ms"""
                for m in ms:
                    ps = ps3.tile([P, 512], F32, name="ps3t")
                    for h in range(HG):
                        nc.tensor.matmul(
                            ps[:, :w], wot_sb[:, h * S + m * P:h * S + (m + 1) * P],
                            ot_sb[h][:, q0:q0 + w],
                            start=(h == 0), stop=(h == HG - 1))
                    st = ost.tile([P, 512], BF16, name="st")
                    nc.vector.tensor_copy(st[:, :w], ps[:, :w])
                    nc.sync.dma_start(
                        out[m, q0 // 512, :, :w], st[:, :w])

            def attn_unit(h, q0, w):
                """causal attention for head h, query cols [q0, q0+w).

                Key tiles processed in PAIRS on 2-bank score tiles: both
                sub-diagonal tiles of a pair share ONE exp (halves ACT's
                352-elem fixed cost); AV/sums of pair u-1 are issued after
                pair u's scores so the PE pipelines across the exp latency.
                """
                nkt = (q0 + w) // P          # key tiles 0..nkt-1; always even
                po = otp.tile([P, 512], F32, name="po")
                sums = smps.tile([P, 512], F32, name="sums")
                pend = []                    # (kt, pcp, region, off) to flush

                def flush():
                    for (kt, pcp, reg, off) in pend:
                        nc.tensor.matmul(
                            po[:, off:w], v_sb[kt][:, h * P:(h + 1) * P],
                            pcp[:, reg, off:w],
                            start=(kt == 0), stop=(kt == nkt - 1),
                            skip_group_check=True)
                        nc.tensor.matmul(
                            sums[:, off:w], ones_sb[:], pcp[:, reg, off:w],
                            start=(kt == 0), stop=(kt == nkt - 1),
                            skip_group_check=True)
                    pend.clear()

                for u in range(nkt // 2):
                    k0, k1 = 2 * u, 2 * u + 1
                    ko0, ko1 = k0 * P - q0, k1 * P - q0
                    m0, m1 = max(0, ko0), max(0, ko1)
                    spb = sp.tile([P, 2, 512], F32, name="ps")
                    nc.tensor.matmul(
                        spb[:, 0, m0:w], kt_sb[h][:, k0 * P:(k0 + 1) * P],
                        qt_sb[h][:, q0 + m0:q0 + w], start=True, stop=True)
                    nc.tensor.matmul(
                        spb[:, 1, m1:w], kt_sb[h][:, k1 * P:(k1 + 1) * P],
                        qt_sb[h][:, q0 + m1:q0 + w], start=True, stop=True)
                    pcp = pp.tile([P, 2, 512], BF16, name="pcp")
                    if ko1 < 0:
                        # both sub-diagonal: one merged exp for the pair
                        nc.scalar.activation(pcp[:, :, :w], spb[:, :, :w],
                                             AF.Exp)
                    else:
                        # diagonal band: triangular mask per tile, then
                        # separate exps over the unmasked column ranges
                        for reg, ko in ((0, ko0), (1, ko1)):
                            off = max(0, ko)
                            if ko >= 0:
                                nc.vector.tensor_add(
                                    spb[:, reg, ko:ko + P],
                                    spb[:, reg, ko:ko + P], mask_sb[:])
                            nc.scalar.activation(pcp[:, reg, off:w],
                                                 spb[:, reg, off:w], AF.Exp)
                    flush()
                    pend.append((k0, pcp, 0, m0))
                    pend.append((k1, pcp, 1, m1))
                flush()
                # 1/x as exp(-ln(x)) on ACT: ~1.3us vs 3.4us DVE divide,
                # and off the DVE critical path (sums are always > 0).
                lg = smp.tile([P, 512], F32, name="lg")
                nc.scalar.activation(lg[:, :w], sums[:, :w], AF.Ln)
                rec = smp.tile([P, 512], F32, name="rec")
                nc.scalar.activation(rec[:, :w], lg[:, :w], AF.Exp, scale=-1.0)
                nc.vector.tensor_mul(ot_sb[h][:, q0:q0 + w],
                                     po[:, :w], rec[:, :w])

            # g-major, h-minor: adjacent (h,g) units are independent heads, so
            # the PE always has a second stream to fill softmax-latency gaps.
            # Between head units: chunk 0 fills with deferred V tiles, later
            # chunks with the previous chunk's output projection.
            for g in range(NC - 1):
                for h in range(HG):
                    attn_unit(h, g * 512, 512)
                    if g == 0:
                        for t in range(4 + 3 * h, 7 + 3 * h):
                            vgroup(t)
                    else:
                        outproj((g - 1) * 512, 512, range(h * 4, (h + 1) * 4))
            # last chunk as two 256-wide subchunks: the first subchunk's
            # output projection overlaps the second subchunk's attention.
            # Both subchunks' results collect in persistent [P,512] tiles;
            # one fat DMA per m-tile at the end (1KB lines, fewer packets).
            stf = [stp.tile([P, 512], BF16, name=f"stf{m}") for m in range(ND)]

            def outproj_tail(q0, half, ms):
                for m in ms:
                    ps = ps3.tile([P, 512], F32, name="ps3t")
                    for h in range(HG):
                        nc.tensor.matmul(
                            ps[:, :256],
                            wot_sb[:, h * S + m * P:h * S + (m + 1) * P],
                            ot_sb[h][:, q0:q0 + 256],
                            start=(h == 0), stop=(h == HG - 1))
                    nc.vector.tensor_copy(
                        stf[m][:, half * 256:(half + 1) * 256], ps[:, :256])
                    if half == 1:
                        nc.sync.dma_start(out[m, 3, :, :], stf[m][:])

            for h in range(HG):
                attn_unit(h, 1536, 256)
                outproj(1024, 512, range(h * 4, (h + 1) * 4))
            for h in range(HG):
                attn_unit(h, 1792, 256)
                outproj_tail(1536, 0, range(h * 4, (h + 1) * 4))
            outproj_tail(1792, 1, range(ND))

    return nc


_NC_CACHE = None


def _get_nc():
    global _NC_CACHE
    if _NC_CACHE is None:
        _NC_CACHE = build_bass()
    return _NC_CACHE


def _prep_core_inputs(x, w_q, b_q, w_k, b_k, w_v, b_v, w_o, b_o, b, c):
    """Host-side shard prep for core (batch b, head-group c)."""
    hsl = slice(c * DHG, (c + 1) * DHG)
    scale = np.float32(1.0 / np.sqrt(DK))

    def pmajor(wt):
        # wt: [D, DHG] (model-major) -> [P, ND*DHG] partition-major
        return np.ascontiguousarray(
            wt.reshape(ND, P, DHG).transpose(1, 0, 2).reshape(P, ND * DHG))

    xb = x[b].T                       # [D, S] f32
    wkt = w_k[hsl].T                  # [D, DHG] f32
    # fp8 scaling: K path computes (x/sx)@(wk/sw); the sx*sw factor is
    # folded into the Q weights/bias (scores = q^T k is scale-invariant)
    # and into b_k (added to the scaled psum at eviction).
    sx = np.float32(np.abs(xb).max() / 448.0 * 2.0)
    sw = np.float32(np.abs(wkt).max() / 448.0 * 2.0)
    sxw = np.float32(sx * sw)

    xtn = np.ascontiguousarray(xb).astype(BF)
    x8n = np.ascontiguousarray(
        (xb / sx).astype(E4).reshape(NP, 2, P, S).transpose(0, 2, 1, 3))
    wk8n = np.ascontiguousarray(
        (wkt / sw).astype(E4).reshape(NP, 2, P, DHG).transpose(2, 0, 1, 3))
    wqtn = pmajor((w_q[hsl] * (scale * sxw)).T.astype(BF))
    wvtn = pmajor(w_v[hsl].T.astype(BF))
    # w_o slice: [DHG, D]; wot2[p, h*S + m] = w_o[m, c*DHG + h*P + p]
    wotn = np.ascontiguousarray(
        w_o[:, hsl].T.astype(BF).reshape(HG, P, D).transpose(1, 0, 2)
        .reshape(P, HG * D))
    bqtn = np.ascontiguousarray(
        (b_q[hsl] * (scale * sxw)).reshape(HG, P).T).astype(np.float32)
    bktn = np.ascontiguousarray(
        (b_k[hsl] / sxw).reshape(HG, P).T).astype(np.float32)
    bvbn = np.ascontiguousarray(np.tile(b_v[hsl], (P, 1))).astype(np.float32)
    i = np.arange(P)[:, None]
    j = np.arange(P)[None, :]
    dmaskn = np.where(j >= i, np.float32(0.0), np.float32(NEG)).astype(np.float32)
    return {
        "xt": xtn, "x8": x8n, "wk8": wk8n, "wq2": wqtn, "wv2": wvtn,
        "wot2": wotn, "bqt": bqtn, "bkt": bktn, "bvb": bvbn, "dmask": dmaskn,
    }


def kernel(x, w_q, b_q, w_k, b_k, w_v, b_v, w_o, b_o, *,
           _trace=False, _tmpdir=None):
    _install_ntff_hook_shim()
    from concourse.bass_utils import run_bass_kernel_spmd

    x = np.asarray(x, dtype=np.float32)
    w_q = np.asarray(w_q, dtype=np.float32)
    b_q = np.asarray(b_q, dtype=np.float32)
    w_k = np.asarray(w_k, dtype=np.float32)
    b_k = np.asarray(b_k, dtype=np.float32)
    w_v = np.asarray(w_v, dtype=np.float32)
    b_v = np.asarray(b_v, dtype=np.float32)
    w_o = np.asarray(w_o, dtype=np.float32)
    b_o = np.asarray(b_o, dtype=np.float32)

    nc = _get_nc()
    in_maps = []
    for core in range(8):
        b, c = divmod(core, 4)
        in_maps.append(_prep_core_inputs(x, w_q, b_q, w_k, b_k, w_v, b_v,
                                         w_o, b_o, b, c))
    kwargs = {}
    if _trace:
        kwargs.update(trace=True, tmpdir=_tmpdir)
    res = run_bass_kernel_spmd(nc, in_maps, core_ids=list(range(8)), **kwargs)

    B = x.shape[0]
    outp = np.zeros((B, S, D), dtype=np.float32)
    for core in range(8):
        b, c = divmod(core, 4)
        blk = res.results[core]["out"]  # [ND, NC, P, 512] tile blocks
        full = blk.transpose(0, 2, 1, 3).reshape(D, S)
        outp[b] += full.T.astype(np.float32)
    outp += b_o[None, None, :]
    kernel.last_results = res
    return outp
